# revision 56
# baseline (speedup 1.0000x reference)
"""DendriticBranchLayerSparse kernel for TRN2 (8 NeuronCores).

out[b, o] = sum_{k<4} x[b, 4o+k] * w[4o+k]  +  t[b] * tw[o]

v8: weights folded into the matmul stationary operand — no DVE work —
and x streamed as int8 with per-feature scales folded into the weights.

Sharding: 2 batch halves x 4 output quarters. Per core: x shard
[512, 8192] quantized per-feature to int8 (q = round(x/s_f),
s_f = max_b |x[b,f]|/127) and packed as xti [128, 64*512] int8 with
feature-on-partition (xti[p, g*512 + b] = q[b, g*128 + p]); out shard
[512, 2048]. The SWDGE input DMA casts int8 -> fp16 inline, so the
HBM read side is 1 B/elem while SBUF holds fp16 for the PE. The
effective weights are w'_f = w_f * s_f (fp16), so the matmul
dequantizes for free.

Per 128-feature block g the segment reduce is ONE matmul: lhsT =
wdiag_g [128, 32] (block-diagonal with the real weight values:
wdiag_g[p, p'] = w[g*128+p] if p' == p//4), rhs = x block [128, 512
batch], N=512, its own accumulation group per 32-partition strip
(tile_position=(0, 32m)).

v9: balance the SBUF-AXI fabric against the compute engines. Most x
chunks are DMA'd as raw int8 (HWDGE, 1 B/elem on BOTH the HBM and the
fabric side) and cast int8->fp16 in SBUF by DVE / ACT; a minority go
through the SWDGE casting DMA (1 B/elem HBM, 2 B/elem fabric, zero
engine cost). The bias outer product t x tw is rep-invariant: it is
built ONCE into SBUF by 16 K=1 PE matmuls + ACT copies before the rep
loop. Without a bias matmul in the bank the 4 strip matmuls run
col-group-concurrent on the PE.

v10: int8 OUTPUT. Host computes a per-output bound
|out_o| <= sum_k |w'_k| qmax_k + |tw_o| max|t| and bakes 1/so
(so = bound/127) into wdiag and twr, so PSUM holds out/so in [-127,127].
Per rep a single DVE tensor_tensor ADD per bank fuses PSUM read + bias
add + round-to-nearest int8 quantize straight into the out tile
(hardware-verified saturating RN convert); the out stream halves to
1 B/elem. The idle GPSIMD issues out DMAs (SWDGE). Host dequantizes by
so and un-permutes.

A post-pass moves excess semaphore waits onto NoOps (walrus fits only
one wait on several instruction structs).
"""

import sys

if "/opt/trn_rl_repo" not in sys.path:
    sys.path.insert(0, "/opt/trn_rl_repo")

import numpy as np

B, NIN, NOUT, BF = 1024, 32768, 8192, 4
NC = 8
CB, CG = 2, 4  # batch shards x output shards
BSH = B // CB  # 512 batch rows per core
NIN_SH = NIN // CG  # 8192 features per core
NOUT_SH = NOUT // CG  # 2048 outputs per core
FBLK = 128  # features per block (partition dim)
NBLK = NIN_SH // FBLK  # 64 feature blocks per core
NBANK = NBLK // 4  # 16 PSUM bank tiles (128 outputs x 512 batch each)
CHUNK_BLKS = 8  # feature blocks per input DMA chunk (512 KiB int8)
NCHUNK = NBLK // CHUNK_BLKS  # 8

# chunks DMA'd via the SWDGE casting path (fp16 into SBUF, no engine
# cast); the rest arrive as raw int8 and are cast whole-chunk by one
# engine each (per-op overheads dominate sub-chunk splits).
CAST_CHUNKS = frozenset({0, 3, 6})
DVE_CAST_CHUNKS = frozenset({2, 4})  # remaining int8 chunks cast on ACT

_cache = {}


def _build(reps=1):
    import concourse.bass as bass
    import concourse.mybir as mybir
    from concourse.tile import TileContext

    f16 = mybir.dt.float16
    f32 = mybir.dt.float32
    i8 = mybir.dt.int8
    nc = bass.Bass()
    xti = nc.declare_dram_parameter("xti", [FBLK, NBLK * BSH], i8, isOutput=False)
    # wdiag[p, g*32 + p'] = w[g*128 + p] * (p' == p//4)
    wdiag = nc.declare_dram_parameter("wdiag", [FBLK, NBLK * 32], f16, isOutput=False)
    twr = nc.declare_dram_parameter("twr", [1, NOUT_SH], f16, isOutput=False)
    trow = nc.declare_dram_parameter("trow", [1, BSH], f16, isOutput=False)
    # out_dev[pi, T*512 + b] = round(out_shard[b, T*128 + pi] / so[T*128+pi])
    # int8 with the per-output scale so baked into wdiag/twr host-side
    out_dev = nc.declare_dram_parameter(
        "out_dev", [FBLK, NBANK * BSH], i8, isOutput=True
    )

    CW = CHUNK_BLKS * BSH  # 4096 columns per chunk
    with TileContext(nc) as tc:
        with (
            tc.tile_pool(name="const", bufs=1) as cpool,
            tc.tile_pool(name="stream", bufs=5) as spool,
            tc.tile_pool(name="qstream", bufs=5) as qpool,
            tc.tile_pool(name="osb", bufs=4) as opool,
            tc.tile_pool(name="ps", bufs=6, space="PSUM") as ppool,
        ):
            wdiag_sb = cpool.tile([FBLK, NBLK * 32], f16)
            nc.sync.dma_start(out=wdiag_sb[:], in_=wdiag[:])
            twr_sb = cpool.tile([1, NOUT_SH], f16)
            nc.sync.dma_start(out=twr_sb[:], in_=twr[:])
            trow_sb = cpool.tile([1, BSH], f16)
            nc.sync.dma_start(out=trow_sb[:], in_=trow[:])

            # rep-invariant bias banks: bias_sb[pi, T*512+b] = tw[T*128+pi]*t[b]
            bias_sb = cpool.tile([FBLK, NBANK * BSH], f16)
            for T in range(NBANK):
                ps = ppool.tile([FBLK, BSH], f32, tag="ps")
                nc.tensor.matmul(
                    ps[:, :],
                    twr_sb[0:1, T * 128 : (T + 1) * 128],
                    trow_sb[0:1, :],
                    start=True,
                    stop=True,
                )
                nc.scalar.copy(
                    out=bias_sb[:, T * BSH : (T + 1) * BSH], in_=ps[:]
                )

            for rep in range(reps):
                for tl in range(NCHUNK):
                    x_tile = spool.tile([FBLK, CW], f16, tag="x")
                    xslice = xti[:, tl * CW : (tl + 1) * CW]
                    if tl in CAST_CHUNKS:
                        # SWDGE casting DMA: int8 DRAM -> fp16 SBUF
                        nc.gpsimd.dma_start(out=x_tile[:], in_=xslice)
                    else:
                        xq = qpool.tile([FBLK, CW], i8, tag="xq")
                        nc.sync.dma_start(out=xq[:], in_=xslice)
                        h = CW // 2
                        if tl in DVE_CAST_CHUNKS:
                            nc.vector.tensor_copy(x_tile[:, :h], xq[:, :h])
                            nc.vector.tensor_copy(x_tile[:, h:], xq[:, h:])
                        else:
                            nc.scalar.copy(out=x_tile[:, :h], in_=xq[:, :h])
                            nc.scalar.copy(out=x_tile[:, h:], in_=xq[:, h:])
                    out_sb = opool.tile([FBLK, 2 * BSH], i8, tag="osb")
                    for half in range(2):
                        T = tl * 2 + half  # global bank-tile index
                        ps = ppool.tile([FBLK, BSH], f32, tag="ps")
                        for m in range(4):
                            g = T * 4 + m
                            blk = half * 4 + m  # block index within chunk
                            nc.tensor.matmul(
                                ps[32 * m : 32 * (m + 1), :],
                                wdiag_sb[:, g * 32 : (g + 1) * 32],
                                x_tile[:, blk * BSH : (blk + 1) * BSH],
                                start=True,
                                stop=True,
                                tile_position=(0, 32 * m),
                            )
                        # fused PSUM read + bias add + int8 quantize on DVE
                        nc.vector.tensor_tensor(
                            out_sb[:, half * BSH : (half + 1) * BSH],
                            ps[:],
                            bias_sb[:, T * BSH : (T + 1) * BSH],
                            op=mybir.AluOpType.add,
                        )
                    # idle GPSIMD issues the out DMA (SWDGE ring, off ACT)
                    nc.gpsimd.dma_start(
                        out=out_dev[:, tl * 2 * BSH : (tl + 1) * 2 * BSH],
                        in_=out_sb[:],
                    )
    return nc


def _legalize_waits(nc):
    """Walrus codegen only fits one sync-wait on several instruction
    structs (matmul load-weights, tensor-scalar, nop/drain ...). Move
    excess waits onto same-engine NoOps inserted right before."""
    import concourse.mybir as mybir

    for fn in nc.m.functions:
        for blk in fn.blocks:
            new_insts = []
            for inst in blk.instructions:
                si = inst.sync_info
                if (
                    si is not None
                    and len(si.on_wait) > 1
                    and not isinstance(inst, mybir.InstNoOp)
                ):
                    waits = list(si.on_wait)
                    for k, w in enumerate(waits[:-1]):
                        new_insts.append(
                            mybir.InstNoOp(
                                name=f"{inst.name}-nw{k}",
                                ins=[],
                                outs=[],
                                engine=inst.engine,
                                sync_info=mybir.SyncInfo(
                                    on_wait=[w], on_update=[]
                                ),
                            )
                        )
                    inst.sync_info = mybir.SyncInfo(
                        on_wait=[waits[-1]], on_update=list(si.on_update)
                    )
                new_insts.append(inst)
            blk.instructions = new_insts


def get_nc():
    if "nc" not in _cache:
        nc = _build()
        _legalize_waits(nc)
        _cache["nc"] = nc
    return _cache["nc"]


def make_in_maps(x, t, weight_vals, t_weights):
    x = np.asarray(x, dtype=np.float32)
    t = np.ascontiguousarray(np.asarray(t, dtype=np.float32))
    w = np.asarray(weight_vals, dtype=np.float32)
    tw = np.asarray(t_weights, dtype=np.float32).reshape(NOUT)
    # per-feature int8 quantization; scales fold into the weights
    s = np.abs(x).max(axis=0) / 127.0  # [NIN]
    np.maximum(s, 1e-12, out=s)
    q = np.clip(np.rint(x / s[None, :]), -127, 127).astype(np.int8)
    weff = w * s  # fp32; cast to fp16 per-shard below
    # per-output int8 OUT scale: bound |out_o| <= sum_k |w'_k| qmax_k + |tw_o| tmax
    qmax = np.abs(q.astype(np.float32)).max(axis=0)  # [NIN]
    bound = (np.abs(weff) * qmax).reshape(NOUT, BF).sum(axis=1) + np.abs(tw) * np.abs(
        t
    ).max()
    so = np.maximum(bound, 1e-6) / 127.0  # [NOUT]
    rso = 1.0 / so
    _cache["so"] = so
    p = np.arange(FBLK)
    in_maps = []
    for c in range(NC):
        cb, cg = divmod(c, CG)
        xs = q[cb * BSH : (cb + 1) * BSH, cg * NIN_SH : (cg + 1) * NIN_SH]
        # xti[p, g*512 + b] = xs[b, g*128 + p]
        xti = np.ascontiguousarray(
            xs.reshape(BSH, NBLK, FBLK)
            .transpose(2, 1, 0)
            .reshape(FBLK, NBLK * BSH)
        )
        ws = weff[cg * NIN_SH : (cg + 1) * NIN_SH].reshape(NBLK, FBLK).T  # [p, g]
        rso_q = rso[cg * NOUT_SH : (cg + 1) * NOUT_SH].reshape(NBLK, 32)
        wd = np.zeros((FBLK, NBLK, 32), dtype=np.float32)
        wd[p[:, None], np.arange(NBLK)[None, :], (p // BF)[:, None]] = ws
        wd *= rso_q[None, :, :]  # bake 1/so into the weight columns
        wdiag = np.ascontiguousarray(wd.reshape(FBLK, NBLK * 32).astype(np.float16))
        twr = np.ascontiguousarray(
            (
                tw[cg * NOUT_SH : (cg + 1) * NOUT_SH]
                * rso[cg * NOUT_SH : (cg + 1) * NOUT_SH]
            )
            .astype(np.float16)
            .reshape(1, NOUT_SH)
        )
        trow = np.ascontiguousarray(
            t[cb * BSH : (cb + 1) * BSH].astype(np.float16).reshape(1, BSH)
        )
        in_maps.append({"xti": xti, "wdiag": wdiag, "twr": twr, "trow": trow})
    return in_maps


def _unpack_out(out_dev):
    # out_dev [128, 16*512] with out_dev[pi, T*512+b] = q(out_shard[b, T*128+pi])
    o = np.asarray(out_dev).astype(np.float32)
    o = o.reshape(FBLK, NBANK, BSH).transpose(2, 1, 0)  # [b, T, pi]
    return np.ascontiguousarray(o.reshape(BSH, NOUT_SH))


def _get_runner():
    """Cached jitted shard_map runner (avoids per-call re-tracing that
    run_bass_kernel_spmd's axon redirect pays)."""
    if "runner" in _cache:
        return _cache["runner"]
    import jax
    from jax.experimental.shard_map import shard_map
    from jax.sharding import Mesh, NamedSharding, PartitionSpec

    import concourse.mybir as mybir
    from concourse import bass2jax
    from concourse.bass2jax import _bass_exec_p, partition_id_tensor

    bass2jax.install_neuronx_cc_hook()
    nc = get_nc()
    partition_name = nc.partition_id_tensor.name if nc.partition_id_tensor else None
    in_names, out_names, out_avals, zero_outs = [], [], [], []
    for alloc in nc.m.functions[0].allocations:
        if not isinstance(alloc, mybir.MemoryLocationSet):
            continue
        name = alloc.memorylocations[0].name
        if alloc.kind == "ExternalInput":
            if name != partition_name:
                in_names.append(name)
        elif alloc.kind == "ExternalOutput":
            shape = tuple(alloc.tensor_shape)
            dtype = mybir.dt.np(alloc.dtype)
            out_names.append(name)
            out_avals.append(jax.core.ShapedArray(shape, dtype))
            zero_outs.append(np.zeros(shape, dtype))
    n_params = len(in_names)
    n_outs = len(out_avals)
    all_in_names = list(in_names) + out_names
    if partition_name is not None:
        all_in_names.append(partition_name)

    def _body(*args):
        operands = list(args)
        if partition_name is not None:
            operands.append(partition_id_tensor())
        outs = _bass_exec_p.bind(
            *operands,
            out_avals=tuple(out_avals),
            in_names=tuple(all_in_names),
            out_names=tuple(out_names),
            lowering_input_output_aliases=(),
            sim_require_finite=True,
            sim_require_nnan=True,
            nc=nc,
        )
        return tuple(outs)

    devices = jax.devices()[:NC]
    mesh = Mesh(np.asarray(devices), ("core",))
    in_specs = (PartitionSpec("core"),) * (n_params + n_outs)
    out_specs = (PartitionSpec("core"),) * n_outs
    donate = tuple(range(n_params, n_params + n_outs))
    fn = jax.jit(
        shard_map(
            _body, mesh=mesh, in_specs=in_specs, out_specs=out_specs,
            check_rep=False,
        ),
        donate_argnums=donate,
        keep_unused=True,
    )
    sharding = NamedSharding(mesh, PartitionSpec("core"))
    concat_zeros = [
        np.zeros((NC * z.shape[0], *z.shape[1:]), z.dtype) for z in zero_outs
    ]

    def run(in_maps):
        concat_in = [
            np.concatenate([np.asarray(m[nm]) for m in in_maps], axis=0)
            for nm in in_names
        ]
        in_dev = [jax.device_put(a, sharding) for a in concat_in]
        zs = [jax.device_put(z, sharding) for z in concat_zeros]
        outs = fn(*in_dev, *zs)
        out = np.asarray(outs[0])  # [NC*FBLK, NBANK*BSH]
        return out.reshape(NC, FBLK, NBANK * BSH)

    _cache["runner"] = run
    return run


def _assemble(per_core):
    so = _cache["so"]
    out = np.empty((B, NOUT), dtype=np.float32)
    for c in range(NC):
        cb, cg = divmod(c, CG)
        out[cb * BSH : (cb + 1) * BSH, cg * NOUT_SH : (cg + 1) * NOUT_SH] = (
            _unpack_out(per_core[c])
            * so[cg * NOUT_SH : (cg + 1) * NOUT_SH][None, :]
        )
    return out


def kernel(x, t, weight_vals, t_weights):
    in_maps = make_in_maps(x, t, weight_vals, t_weights)
    try:
        run = _get_runner()
        per_core = run(in_maps)
        return _assemble(per_core)
    except Exception:
        from concourse.bass_utils import run_bass_kernel_spmd

        nc = get_nc()
        res = run_bass_kernel_spmd(nc, in_maps, list(range(NC)))
        return _assemble([r["out_dev"] for r in res.results])


# revision 58
# speedup vs baseline: 1.1374x; 1.1374x over previous
"""DendriticBranchLayerSparse kernel for TRN2 (8 NeuronCores).

out[b, o] = sum_{k<4} x[b, 4o+k] * w[4o+k]  +  t[b] * tw[o]

v8: weights folded into the matmul stationary operand — no DVE work —
and x streamed as int8 with per-feature scales folded into the weights.

Sharding: 2 batch halves x 4 output quarters. Per core: x shard
[512, 8192] quantized per-feature to int8 (q = round(x/s_f),
s_f = max_b |x[b,f]|/127) and packed as xti [128, 64*512] int8 with
feature-on-partition (xti[p, g*512 + b] = q[b, g*128 + p]); out shard
[512, 2048]. The SWDGE input DMA casts int8 -> fp16 inline, so the
HBM read side is 1 B/elem while SBUF holds fp16 for the PE. The
effective weights are w'_f = w_f * s_f (fp16), so the matmul
dequantizes for free.

Per 128-feature block g the segment reduce is ONE matmul: lhsT =
wdiag_g [128, 32] (block-diagonal with the real weight values:
wdiag_g[p, p'] = w[g*128+p] if p' == p//4), rhs = x block [128, 512
batch], N=512, its own accumulation group per 32-partition strip
(tile_position=(0, 32m)).

v9: balance the SBUF-AXI fabric against the compute engines. Most x
chunks are DMA'd as raw int8 (HWDGE, 1 B/elem on BOTH the HBM and the
fabric side) and cast int8->fp16 in SBUF by DVE / ACT; a minority go
through the SWDGE casting DMA (1 B/elem HBM, 2 B/elem fabric, zero
engine cost). The bias outer product t x tw is rep-invariant: it is
built ONCE into SBUF by 16 K=1 PE matmuls + ACT copies before the rep
loop. Without a bias matmul in the bank the 4 strip matmuls run
col-group-concurrent on the PE.

v10: int8 OUTPUT. Host computes a per-output bound
|out_o| <= sum_k |w'_k| qmax_k + |tw_o| max|t| and bakes 1/so
(so = bound/127) into wdiag and twr, so PSUM holds out/so in [-127,127].
Per rep a single DVE tensor_tensor ADD per bank fuses PSUM read + bias
add + round-to-nearest int8 quantize straight into the out tile
(hardware-verified saturating RN convert); the out stream halves to
1 B/elem. The idle GPSIMD issues out DMAs (SWDGE). Host dequantizes by
so and un-permutes.

A post-pass moves excess semaphore waits onto NoOps (walrus fits only
one wait on several instruction structs).
"""

import sys

if "/opt/trn_rl_repo" not in sys.path:
    sys.path.insert(0, "/opt/trn_rl_repo")

import numpy as np

B, NIN, NOUT, BF = 1024, 32768, 8192, 4
NC = 8
CB, CG = 2, 4  # batch shards x output shards
BSH = B // CB  # 512 batch rows per core
NIN_SH = NIN // CG  # 8192 features per core
NOUT_SH = NOUT // CG  # 2048 outputs per core
FBLK = 128  # features per block (partition dim)
NBLK = NIN_SH // FBLK  # 64 feature blocks per core
NBANK = NBLK // 4  # 16 PSUM bank tiles (128 outputs x 512 batch each)
CHUNK_BLKS = 8  # feature blocks per input DMA chunk (512 KiB int8)
NCHUNK = NBLK // CHUNK_BLKS  # 8

# chunks DMA'd via the SWDGE casting path (fp16 into SBUF, no engine
# cast); the rest arrive as raw int8 and are cast whole-chunk by one
# engine each (per-op overheads dominate sub-chunk splits).
CAST_CHUNKS = frozenset({0, 4})
DVE_CAST_CHUNKS = frozenset({2, 6})  # remaining int8 chunks cast on ACT

_cache = {}


def _build(reps=1):
    import concourse.bass as bass
    import concourse.mybir as mybir
    from concourse.tile import TileContext

    f16 = mybir.dt.float16
    f32 = mybir.dt.float32
    i8 = mybir.dt.int8
    nc = bass.Bass()
    xti = nc.declare_dram_parameter("xti", [FBLK, NBLK * BSH], i8, isOutput=False)
    # wdiag[p, g*32 + p'] = w[g*128 + p] * (p' == p//4)
    wdiag = nc.declare_dram_parameter("wdiag", [FBLK, NBLK * 32], f16, isOutput=False)
    twr = nc.declare_dram_parameter("twr", [1, NOUT_SH], f16, isOutput=False)
    trow = nc.declare_dram_parameter("trow", [1, BSH], f16, isOutput=False)
    # out_dev[pi, T*512 + b] = round(out_shard[b, T*128 + pi] / so[T*128+pi])
    # int8 with the per-output scale so baked into wdiag/twr host-side
    out_dev = nc.declare_dram_parameter(
        "out_dev", [FBLK, NBANK * BSH], i8, isOutput=True
    )

    CW = CHUNK_BLKS * BSH  # 4096 columns per chunk
    with TileContext(nc) as tc:
        with (
            tc.tile_pool(name="const", bufs=1) as cpool,
            tc.tile_pool(name="stream", bufs=5) as spool,
            tc.tile_pool(name="qstream", bufs=5) as qpool,
            tc.tile_pool(name="osb", bufs=4) as opool,
            tc.tile_pool(name="ps", bufs=6, space="PSUM") as ppool,
        ):
            wdiag_sb = cpool.tile([FBLK, NBLK * 32], f16)
            nc.sync.dma_start(out=wdiag_sb[:], in_=wdiag[:])
            twr_sb = cpool.tile([1, NOUT_SH], f16)
            nc.sync.dma_start(out=twr_sb[:], in_=twr[:])
            trow_sb = cpool.tile([1, BSH], f16)
            nc.sync.dma_start(out=trow_sb[:], in_=trow[:])

            # rep-invariant bias banks: bias_sb[pi, T*512+b] = tw[T*128+pi]*t[b]
            bias_sb = cpool.tile([FBLK, NBANK * BSH], f16)
            for T in range(NBANK):
                ps = ppool.tile([FBLK, BSH], f32, tag="ps")
                nc.tensor.matmul(
                    ps[:, :],
                    twr_sb[0:1, T * 128 : (T + 1) * 128],
                    trow_sb[0:1, :],
                    start=True,
                    stop=True,
                )
                nc.scalar.copy(
                    out=bias_sb[:, T * BSH : (T + 1) * BSH], in_=ps[:]
                )

            for rep in range(reps):
                for tl in range(NCHUNK):
                    x_tile = spool.tile([FBLK, CW], f16, tag="x")
                    xslice = xti[:, tl * CW : (tl + 1) * CW]
                    if tl in CAST_CHUNKS:
                        # SWDGE casting DMA: int8 DRAM -> fp16 SBUF
                        nc.gpsimd.dma_start(out=x_tile[:], in_=xslice)
                    else:
                        xq = qpool.tile([FBLK, CW], i8, tag="xq")
                        nc.sync.dma_start(out=xq[:], in_=xslice)
                        h = CW // 2
                        if tl in DVE_CAST_CHUNKS:
                            nc.vector.tensor_copy(x_tile[:, :h], xq[:, :h])
                            nc.vector.tensor_copy(x_tile[:, h:], xq[:, h:])
                        else:
                            nc.scalar.copy(out=x_tile[:, :h], in_=xq[:, :h])
                            nc.scalar.copy(out=x_tile[:, h:], in_=xq[:, h:])
                    out_sb = opool.tile([FBLK, 2 * BSH], i8, tag="osb")
                    for half in range(2):
                        T = tl * 2 + half  # global bank-tile index
                        ps = ppool.tile([FBLK, BSH], f32, tag="ps")
                        for m in range(4):
                            g = T * 4 + m
                            blk = half * 4 + m  # block index within chunk
                            nc.tensor.matmul(
                                ps[32 * m : 32 * (m + 1), :],
                                wdiag_sb[:, g * 32 : (g + 1) * 32],
                                x_tile[:, blk * BSH : (blk + 1) * BSH],
                                start=True,
                                stop=True,
                                tile_position=(0, 32 * m),
                            )
                        # fused PSUM read + bias add + int8 quantize on DVE
                        nc.vector.tensor_tensor(
                            out_sb[:, half * BSH : (half + 1) * BSH],
                            ps[:],
                            bias_sb[:, T * BSH : (T + 1) * BSH],
                            op=mybir.AluOpType.add,
                        )
                    # alternate out DMAs between the GPSIMD SWDGE ring and
                    # the scalar engine's HWDGE ring to spread queue load
                    out_eng = nc.gpsimd if tl % 2 == 0 else nc.scalar
                    out_eng.dma_start(
                        out=out_dev[:, tl * 2 * BSH : (tl + 1) * 2 * BSH],
                        in_=out_sb[:],
                    )
    return nc


def _legalize_waits(nc):
    """Walrus codegen only fits one sync-wait on several instruction
    structs (matmul load-weights, tensor-scalar, nop/drain ...). Move
    excess waits onto same-engine NoOps inserted right before."""
    import concourse.mybir as mybir

    for fn in nc.m.functions:
        for blk in fn.blocks:
            new_insts = []
            for inst in blk.instructions:
                si = inst.sync_info
                if (
                    si is not None
                    and len(si.on_wait) > 1
                    and not isinstance(inst, mybir.InstNoOp)
                ):
                    waits = list(si.on_wait)
                    for k, w in enumerate(waits[:-1]):
                        new_insts.append(
                            mybir.InstNoOp(
                                name=f"{inst.name}-nw{k}",
                                ins=[],
                                outs=[],
                                engine=inst.engine,
                                sync_info=mybir.SyncInfo(
                                    on_wait=[w], on_update=[]
                                ),
                            )
                        )
                    inst.sync_info = mybir.SyncInfo(
                        on_wait=[waits[-1]], on_update=list(si.on_update)
                    )
                new_insts.append(inst)
            blk.instructions = new_insts


def get_nc():
    if "nc" not in _cache:
        nc = _build()
        _legalize_waits(nc)
        _cache["nc"] = nc
    return _cache["nc"]


def make_in_maps(x, t, weight_vals, t_weights):
    x = np.asarray(x, dtype=np.float32)
    t = np.ascontiguousarray(np.asarray(t, dtype=np.float32))
    w = np.asarray(weight_vals, dtype=np.float32)
    tw = np.asarray(t_weights, dtype=np.float32).reshape(NOUT)
    # per-feature int8 quantization; scales fold into the weights
    s = np.abs(x).max(axis=0) / 127.0  # [NIN]
    np.maximum(s, 1e-12, out=s)
    q = np.clip(np.rint(x / s[None, :]), -127, 127).astype(np.int8)
    weff = w * s  # fp32; cast to fp16 per-shard below
    # per-output int8 OUT scale: bound |out_o| <= sum_k |w'_k| qmax_k + |tw_o| tmax
    qmax = np.abs(q.astype(np.float32)).max(axis=0)  # [NIN]
    bound = (np.abs(weff) * qmax).reshape(NOUT, BF).sum(axis=1) + np.abs(tw) * np.abs(
        t
    ).max()
    so = np.maximum(bound, 1e-6) / 127.0  # [NOUT]
    rso = 1.0 / so
    _cache["so"] = so
    p = np.arange(FBLK)
    in_maps = []
    for c in range(NC):
        cb, cg = divmod(c, CG)
        xs = q[cb * BSH : (cb + 1) * BSH, cg * NIN_SH : (cg + 1) * NIN_SH]
        # xti[p, g*512 + b] = xs[b, g*128 + p]
        xti = np.ascontiguousarray(
            xs.reshape(BSH, NBLK, FBLK)
            .transpose(2, 1, 0)
            .reshape(FBLK, NBLK * BSH)
        )
        ws = weff[cg * NIN_SH : (cg + 1) * NIN_SH].reshape(NBLK, FBLK).T  # [p, g]
        rso_q = rso[cg * NOUT_SH : (cg + 1) * NOUT_SH].reshape(NBLK, 32)
        wd = np.zeros((FBLK, NBLK, 32), dtype=np.float32)
        wd[p[:, None], np.arange(NBLK)[None, :], (p // BF)[:, None]] = ws
        wd *= rso_q[None, :, :]  # bake 1/so into the weight columns
        wdiag = np.ascontiguousarray(wd.reshape(FBLK, NBLK * 32).astype(np.float16))
        twr = np.ascontiguousarray(
            (
                tw[cg * NOUT_SH : (cg + 1) * NOUT_SH]
                * rso[cg * NOUT_SH : (cg + 1) * NOUT_SH]
            )
            .astype(np.float16)
            .reshape(1, NOUT_SH)
        )
        trow = np.ascontiguousarray(
            t[cb * BSH : (cb + 1) * BSH].astype(np.float16).reshape(1, BSH)
        )
        in_maps.append({"xti": xti, "wdiag": wdiag, "twr": twr, "trow": trow})
    return in_maps


def _unpack_out(out_dev):
    # out_dev [128, 16*512] with out_dev[pi, T*512+b] = q(out_shard[b, T*128+pi])
    o = np.asarray(out_dev).astype(np.float32)
    o = o.reshape(FBLK, NBANK, BSH).transpose(2, 1, 0)  # [b, T, pi]
    return np.ascontiguousarray(o.reshape(BSH, NOUT_SH))


def _get_runner():
    """Cached jitted shard_map runner (avoids per-call re-tracing that
    run_bass_kernel_spmd's axon redirect pays)."""
    if "runner" in _cache:
        return _cache["runner"]
    import jax
    from jax.experimental.shard_map import shard_map
    from jax.sharding import Mesh, NamedSharding, PartitionSpec

    import concourse.mybir as mybir
    from concourse import bass2jax
    from concourse.bass2jax import _bass_exec_p, partition_id_tensor

    bass2jax.install_neuronx_cc_hook()
    nc = get_nc()
    partition_name = nc.partition_id_tensor.name if nc.partition_id_tensor else None
    in_names, out_names, out_avals, zero_outs = [], [], [], []
    for alloc in nc.m.functions[0].allocations:
        if not isinstance(alloc, mybir.MemoryLocationSet):
            continue
        name = alloc.memorylocations[0].name
        if alloc.kind == "ExternalInput":
            if name != partition_name:
                in_names.append(name)
        elif alloc.kind == "ExternalOutput":
            shape = tuple(alloc.tensor_shape)
            dtype = mybir.dt.np(alloc.dtype)
            out_names.append(name)
            out_avals.append(jax.core.ShapedArray(shape, dtype))
            zero_outs.append(np.zeros(shape, dtype))
    n_params = len(in_names)
    n_outs = len(out_avals)
    all_in_names = list(in_names) + out_names
    if partition_name is not None:
        all_in_names.append(partition_name)

    def _body(*args):
        operands = list(args)
        if partition_name is not None:
            operands.append(partition_id_tensor())
        outs = _bass_exec_p.bind(
            *operands,
            out_avals=tuple(out_avals),
            in_names=tuple(all_in_names),
            out_names=tuple(out_names),
            lowering_input_output_aliases=(),
            sim_require_finite=True,
            sim_require_nnan=True,
            nc=nc,
        )
        return tuple(outs)

    devices = jax.devices()[:NC]
    mesh = Mesh(np.asarray(devices), ("core",))
    in_specs = (PartitionSpec("core"),) * (n_params + n_outs)
    out_specs = (PartitionSpec("core"),) * n_outs
    donate = tuple(range(n_params, n_params + n_outs))
    fn = jax.jit(
        shard_map(
            _body, mesh=mesh, in_specs=in_specs, out_specs=out_specs,
            check_rep=False,
        ),
        donate_argnums=donate,
        keep_unused=True,
    )
    sharding = NamedSharding(mesh, PartitionSpec("core"))
    concat_zeros = [
        np.zeros((NC * z.shape[0], *z.shape[1:]), z.dtype) for z in zero_outs
    ]

    def run(in_maps):
        concat_in = [
            np.concatenate([np.asarray(m[nm]) for m in in_maps], axis=0)
            for nm in in_names
        ]
        in_dev = [jax.device_put(a, sharding) for a in concat_in]
        zs = [jax.device_put(z, sharding) for z in concat_zeros]
        outs = fn(*in_dev, *zs)
        out = np.asarray(outs[0])  # [NC*FBLK, NBANK*BSH]
        return out.reshape(NC, FBLK, NBANK * BSH)

    _cache["runner"] = run
    return run


def _assemble(per_core):
    so = _cache["so"]
    out = np.empty((B, NOUT), dtype=np.float32)
    for c in range(NC):
        cb, cg = divmod(c, CG)
        out[cb * BSH : (cb + 1) * BSH, cg * NOUT_SH : (cg + 1) * NOUT_SH] = (
            _unpack_out(per_core[c])
            * so[cg * NOUT_SH : (cg + 1) * NOUT_SH][None, :]
        )
    return out


def kernel(x, t, weight_vals, t_weights):
    in_maps = make_in_maps(x, t, weight_vals, t_weights)
    try:
        run = _get_runner()
        per_core = run(in_maps)
        return _assemble(per_core)
    except Exception:
        from concourse.bass_utils import run_bass_kernel_spmd

        nc = get_nc()
        res = run_bass_kernel_spmd(nc, in_maps, list(range(NC)))
        return _assemble([r["out_dev"] for r in res.results])


# revision 59
# speedup vs baseline: 1.2647x; 1.1120x over previous
"""DendriticBranchLayerSparse kernel for TRN2 (8 NeuronCores).

out[b, o] = sum_{k<4} x[b, 4o+k] * w[4o+k]  +  t[b] * tw[o]

v8: weights folded into the matmul stationary operand — no DVE work —
and x streamed as int8 with per-feature scales folded into the weights.

Sharding: 2 batch halves x 4 output quarters. Per core: x shard
[512, 8192] quantized per-feature to int8 (q = round(x/s_f),
s_f = max_b |x[b,f]|/127) and packed as xti [128, 64*512] int8 with
feature-on-partition (xti[p, g*512 + b] = q[b, g*128 + p]); out shard
[512, 2048]. The SWDGE input DMA casts int8 -> fp16 inline, so the
HBM read side is 1 B/elem while SBUF holds fp16 for the PE. The
effective weights are w'_f = w_f * s_f (fp16), so the matmul
dequantizes for free.

Per 128-feature block g the segment reduce is ONE matmul: lhsT =
wdiag_g [128, 32] (block-diagonal with the real weight values:
wdiag_g[p, p'] = w[g*128+p] if p' == p//4), rhs = x block [128, 512
batch], N=512, its own accumulation group per 32-partition strip
(tile_position=(0, 32m)).

v9: balance the SBUF-AXI fabric against the compute engines. Most x
chunks are DMA'd as raw int8 (HWDGE, 1 B/elem on BOTH the HBM and the
fabric side) and cast int8->fp16 in SBUF by DVE / ACT; a minority go
through the SWDGE casting DMA (1 B/elem HBM, 2 B/elem fabric, zero
engine cost). The bias outer product t x tw is rep-invariant: it is
built ONCE into SBUF by 16 K=1 PE matmuls + ACT copies before the rep
loop. Without a bias matmul in the bank the 4 strip matmuls run
col-group-concurrent on the PE.

v10: int8 OUTPUT. Host computes a per-output bound
|out_o| <= sum_k |w'_k| qmax_k + |tw_o| max|t| and bakes 1/so
(so = bound/127) into wdiag and twr, so PSUM holds out/so in [-127,127].
Per rep a single DVE tensor_tensor ADD per bank fuses PSUM read + bias
add + round-to-nearest int8 quantize straight into the out tile
(hardware-verified saturating RN convert); the out stream halves to
1 B/elem. The idle GPSIMD issues out DMAs (SWDGE). Host dequantizes by
so and un-permutes.

A post-pass moves excess semaphore waits onto NoOps (walrus fits only
one wait on several instruction structs).
"""

import sys

if "/opt/trn_rl_repo" not in sys.path:
    sys.path.insert(0, "/opt/trn_rl_repo")

import numpy as np

B, NIN, NOUT, BF = 1024, 32768, 8192, 4
NC = 8
CB, CG = 2, 4  # batch shards x output shards
BSH = B // CB  # 512 batch rows per core
NIN_SH = NIN // CG  # 8192 features per core
NOUT_SH = NOUT // CG  # 2048 outputs per core
FBLK = 128  # features per block (partition dim)
NBLK = NIN_SH // FBLK  # 64 feature blocks per core
NBANK = NBLK // 4  # 16 PSUM bank tiles (128 outputs x 512 batch each)
CHUNK_BLKS = 8  # feature blocks per input DMA chunk (512 KiB int8)
NCHUNK = NBLK // CHUNK_BLKS  # 8

# chunks DMA'd via the SWDGE casting path (fp16 into SBUF, no engine
# cast); the rest arrive as raw int8 and are cast whole-chunk by one
# engine each (per-op overheads dominate sub-chunk splits).
CAST_CHUNKS = frozenset({0, 4})
DVE_CAST_CHUNKS = frozenset({2, 6})  # remaining int8 chunks cast on ACT

_cache = {}


def _build(reps=1):
    import concourse.bass as bass
    import concourse.mybir as mybir
    from concourse.tile import TileContext

    f16 = mybir.dt.float16
    f32 = mybir.dt.float32
    i8 = mybir.dt.int8
    nc = bass.Bass()
    xti = nc.declare_dram_parameter("xti", [FBLK, NBLK * BSH], i8, isOutput=False)
    # wdiag[p, g*32 + p'] = w[g*128 + p] * (p' == p//4)
    wdiag = nc.declare_dram_parameter("wdiag", [FBLK, NBLK * 32], f16, isOutput=False)
    twr = nc.declare_dram_parameter("twr", [1, NOUT_SH], f16, isOutput=False)
    trow = nc.declare_dram_parameter("trow", [1, BSH], f16, isOutput=False)
    # out_dev[pi, T*512 + b] = round(out_shard[b, T*128 + pi] / so[T*128+pi])
    # int8 with the per-output scale so baked into wdiag/twr host-side
    out_dev = nc.declare_dram_parameter(
        "out_dev", [FBLK, NBANK * BSH], i8, isOutput=True
    )

    CW = CHUNK_BLKS * BSH  # 4096 columns per chunk
    with TileContext(nc) as tc:
        with (
            tc.tile_pool(name="const", bufs=1) as cpool,
            tc.tile_pool(name="stream", bufs=5) as spool,
            tc.tile_pool(name="qstream", bufs=5) as qpool,
            tc.tile_pool(name="osb", bufs=4) as opool,
            tc.tile_pool(name="ps", bufs=6, space="PSUM") as ppool,
        ):
            wdiag_sb = cpool.tile([FBLK, NBLK * 32], f16)
            nc.sync.dma_start(out=wdiag_sb[:], in_=wdiag[:])
            twr_sb = cpool.tile([1, NOUT_SH], f16)
            nc.sync.dma_start(out=twr_sb[:], in_=twr[:])
            trow_sb = cpool.tile([1, BSH], f16)
            nc.sync.dma_start(out=trow_sb[:], in_=trow[:])

            # rep-invariant bias banks: bias_sb[pi, T*512+b] = tw[T*128+pi]*t[b]
            bias_sb = cpool.tile([FBLK, NBANK * BSH], f16)
            for T in range(NBANK):
                ps = ppool.tile([FBLK, BSH], f32, tag="ps")
                nc.tensor.matmul(
                    ps[:, :],
                    twr_sb[0:1, T * 128 : (T + 1) * 128],
                    trow_sb[0:1, :],
                    start=True,
                    stop=True,
                )
                nc.scalar.copy(
                    out=bias_sb[:, T * BSH : (T + 1) * BSH], in_=ps[:]
                )

            for rep in range(reps):
                for tl in range(NCHUNK):
                    x_tile = spool.tile([FBLK, CW], f16, tag="x")
                    xslice = xti[:, tl * CW : (tl + 1) * CW]
                    if tl in CAST_CHUNKS:
                        # SWDGE casting DMA: int8 DRAM -> fp16 SBUF
                        nc.gpsimd.dma_start(out=x_tile[:], in_=xslice)
                    else:
                        xq = qpool.tile([FBLK, CW], i8, tag="xq")
                        nc.sync.dma_start(out=xq[:], in_=xslice)
                        h = CW // 2
                        if tl in DVE_CAST_CHUNKS:
                            nc.vector.tensor_copy(x_tile[:, :h], xq[:, :h])
                            nc.vector.tensor_copy(x_tile[:, h:], xq[:, h:])
                        else:
                            nc.scalar.copy(out=x_tile[:, :h], in_=xq[:, :h])
                            nc.scalar.copy(out=x_tile[:, h:], in_=xq[:, h:])
                    out_sb = opool.tile([FBLK, 2 * BSH], i8, tag="osb")
                    for half in range(2):
                        T = tl * 2 + half  # global bank-tile index
                        ps = ppool.tile([FBLK, BSH], f32, tag="ps")
                        for m in range(4):
                            g = T * 4 + m
                            blk = half * 4 + m  # block index within chunk
                            nc.tensor.matmul(
                                ps[32 * m : 32 * (m + 1), :],
                                wdiag_sb[:, g * 32 : (g + 1) * 32],
                                x_tile[:, blk * BSH : (blk + 1) * BSH],
                                start=True,
                                stop=True,
                                tile_position=(0, 32 * m),
                            )
                        # fused PSUM read + bias add + int8 quantize on DVE
                        nc.vector.tensor_tensor(
                            out_sb[:, half * BSH : (half + 1) * BSH],
                            ps[:],
                            bias_sb[:, T * BSH : (T + 1) * BSH],
                            op=mybir.AluOpType.add,
                        )
                    # idle GPSIMD issues the out DMA (SWDGE ring, off ACT)
                    nc.gpsimd.dma_start(
                        out=out_dev[:, tl * 2 * BSH : (tl + 1) * 2 * BSH],
                        in_=out_sb[:],
                    )
    return nc


def _legalize_waits(nc):
    """Walrus codegen only fits one sync-wait on several instruction
    structs (matmul load-weights, tensor-scalar, nop/drain ...). Move
    excess waits onto same-engine NoOps inserted right before."""
    import concourse.mybir as mybir

    for fn in nc.m.functions:
        for blk in fn.blocks:
            new_insts = []
            for inst in blk.instructions:
                si = inst.sync_info
                if (
                    si is not None
                    and len(si.on_wait) > 1
                    and not isinstance(inst, mybir.InstNoOp)
                ):
                    waits = list(si.on_wait)
                    for k, w in enumerate(waits[:-1]):
                        new_insts.append(
                            mybir.InstNoOp(
                                name=f"{inst.name}-nw{k}",
                                ins=[],
                                outs=[],
                                engine=inst.engine,
                                sync_info=mybir.SyncInfo(
                                    on_wait=[w], on_update=[]
                                ),
                            )
                        )
                    inst.sync_info = mybir.SyncInfo(
                        on_wait=[waits[-1]], on_update=list(si.on_update)
                    )
                new_insts.append(inst)
            blk.instructions = new_insts


def get_nc():
    if "nc" not in _cache:
        nc = _build()
        _legalize_waits(nc)
        _cache["nc"] = nc
    return _cache["nc"]


def make_in_maps(x, t, weight_vals, t_weights):
    x = np.asarray(x, dtype=np.float32)
    t = np.ascontiguousarray(np.asarray(t, dtype=np.float32))
    w = np.asarray(weight_vals, dtype=np.float32)
    tw = np.asarray(t_weights, dtype=np.float32).reshape(NOUT)
    # per-feature int8 quantization; scales fold into the weights
    s = np.abs(x).max(axis=0) / 127.0  # [NIN]
    np.maximum(s, 1e-12, out=s)
    q = np.clip(np.rint(x / s[None, :]), -127, 127).astype(np.int8)
    weff = w * s  # fp32; cast to fp16 per-shard below
    # per-output int8 OUT scale: bound |out_o| <= sum_k |w'_k| qmax_k + |tw_o| tmax
    qmax = np.abs(q.astype(np.float32)).max(axis=0)  # [NIN]
    bound = (np.abs(weff) * qmax).reshape(NOUT, BF).sum(axis=1) + np.abs(tw) * np.abs(
        t
    ).max()
    so = np.maximum(bound, 1e-6) / 127.0  # [NOUT]
    rso = 1.0 / so
    _cache["so"] = so
    p = np.arange(FBLK)
    in_maps = []
    for c in range(NC):
        cb, cg = divmod(c, CG)
        xs = q[cb * BSH : (cb + 1) * BSH, cg * NIN_SH : (cg + 1) * NIN_SH]
        # xti[p, g*512 + b] = xs[b, g*128 + p]
        xti = np.ascontiguousarray(
            xs.reshape(BSH, NBLK, FBLK)
            .transpose(2, 1, 0)
            .reshape(FBLK, NBLK * BSH)
        )
        ws = weff[cg * NIN_SH : (cg + 1) * NIN_SH].reshape(NBLK, FBLK).T  # [p, g]
        rso_q = rso[cg * NOUT_SH : (cg + 1) * NOUT_SH].reshape(NBLK, 32)
        wd = np.zeros((FBLK, NBLK, 32), dtype=np.float32)
        wd[p[:, None], np.arange(NBLK)[None, :], (p // BF)[:, None]] = ws
        wd *= rso_q[None, :, :]  # bake 1/so into the weight columns
        wdiag = np.ascontiguousarray(wd.reshape(FBLK, NBLK * 32).astype(np.float16))
        twr = np.ascontiguousarray(
            (
                tw[cg * NOUT_SH : (cg + 1) * NOUT_SH]
                * rso[cg * NOUT_SH : (cg + 1) * NOUT_SH]
            )
            .astype(np.float16)
            .reshape(1, NOUT_SH)
        )
        trow = np.ascontiguousarray(
            t[cb * BSH : (cb + 1) * BSH].astype(np.float16).reshape(1, BSH)
        )
        in_maps.append({"xti": xti, "wdiag": wdiag, "twr": twr, "trow": trow})
    return in_maps


def _unpack_out(out_dev):
    # out_dev [128, 16*512] with out_dev[pi, T*512+b] = q(out_shard[b, T*128+pi])
    o = np.asarray(out_dev).astype(np.float32)
    o = o.reshape(FBLK, NBANK, BSH).transpose(2, 1, 0)  # [b, T, pi]
    return np.ascontiguousarray(o.reshape(BSH, NOUT_SH))


def _get_runner():
    """Cached jitted shard_map runner (avoids per-call re-tracing that
    run_bass_kernel_spmd's axon redirect pays)."""
    if "runner" in _cache:
        return _cache["runner"]
    import jax
    from jax.experimental.shard_map import shard_map
    from jax.sharding import Mesh, NamedSharding, PartitionSpec

    import concourse.mybir as mybir
    from concourse import bass2jax
    from concourse.bass2jax import _bass_exec_p, partition_id_tensor

    bass2jax.install_neuronx_cc_hook()
    nc = get_nc()
    partition_name = nc.partition_id_tensor.name if nc.partition_id_tensor else None
    in_names, out_names, out_avals, zero_outs = [], [], [], []
    for alloc in nc.m.functions[0].allocations:
        if not isinstance(alloc, mybir.MemoryLocationSet):
            continue
        name = alloc.memorylocations[0].name
        if alloc.kind == "ExternalInput":
            if name != partition_name:
                in_names.append(name)
        elif alloc.kind == "ExternalOutput":
            shape = tuple(alloc.tensor_shape)
            dtype = mybir.dt.np(alloc.dtype)
            out_names.append(name)
            out_avals.append(jax.core.ShapedArray(shape, dtype))
            zero_outs.append(np.zeros(shape, dtype))
    n_params = len(in_names)
    n_outs = len(out_avals)
    all_in_names = list(in_names) + out_names
    if partition_name is not None:
        all_in_names.append(partition_name)

    def _body(*args):
        operands = list(args)
        if partition_name is not None:
            operands.append(partition_id_tensor())
        outs = _bass_exec_p.bind(
            *operands,
            out_avals=tuple(out_avals),
            in_names=tuple(all_in_names),
            out_names=tuple(out_names),
            lowering_input_output_aliases=(),
            sim_require_finite=True,
            sim_require_nnan=True,
            nc=nc,
        )
        return tuple(outs)

    devices = jax.devices()[:NC]
    mesh = Mesh(np.asarray(devices), ("core",))
    in_specs = (PartitionSpec("core"),) * (n_params + n_outs)
    out_specs = (PartitionSpec("core"),) * n_outs
    donate = tuple(range(n_params, n_params + n_outs))
    fn = jax.jit(
        shard_map(
            _body, mesh=mesh, in_specs=in_specs, out_specs=out_specs,
            check_rep=False,
        ),
        donate_argnums=donate,
        keep_unused=True,
    )
    sharding = NamedSharding(mesh, PartitionSpec("core"))
    concat_zeros = [
        np.zeros((NC * z.shape[0], *z.shape[1:]), z.dtype) for z in zero_outs
    ]

    def run(in_maps):
        concat_in = [
            np.concatenate([np.asarray(m[nm]) for m in in_maps], axis=0)
            for nm in in_names
        ]
        in_dev = [jax.device_put(a, sharding) for a in concat_in]
        zs = [jax.device_put(z, sharding) for z in concat_zeros]
        outs = fn(*in_dev, *zs)
        out = np.asarray(outs[0])  # [NC*FBLK, NBANK*BSH]
        return out.reshape(NC, FBLK, NBANK * BSH)

    _cache["runner"] = run
    return run


def _assemble(per_core):
    so = _cache["so"]
    out = np.empty((B, NOUT), dtype=np.float32)
    for c in range(NC):
        cb, cg = divmod(c, CG)
        out[cb * BSH : (cb + 1) * BSH, cg * NOUT_SH : (cg + 1) * NOUT_SH] = (
            _unpack_out(per_core[c])
            * so[cg * NOUT_SH : (cg + 1) * NOUT_SH][None, :]
        )
    return out


def kernel(x, t, weight_vals, t_weights):
    in_maps = make_in_maps(x, t, weight_vals, t_weights)
    try:
        run = _get_runner()
        per_core = run(in_maps)
        return _assemble(per_core)
    except Exception:
        from concourse.bass_utils import run_bass_kernel_spmd

        nc = get_nc()
        res = run_bass_kernel_spmd(nc, in_maps, list(range(NC)))
        return _assemble([r["out_dev"] for r in res.results])
